# revision 1
# baseline (speedup 1.0000x reference)
"""Trainium2 Bass kernel for a transformer decoder layer (pre-norm, eval mode).

Computation (per batch row):
    x = x + MHA(LN1(x), LN1(x), LN1(x), mask)      # masked self-attention
    x = x + MHA(LN2(x), enc, enc, None)            # cross-attention
    x = x + W2 @ relu(W1 @ LN3(x) + b1) + b2       # FFN

Shapes: B=4, S=2048, D=512, H=8 heads (dk=64), FF=1024, fp32.

Sharding: 8 cores = (batch b, query-half). Each core computes 1024 query rows
of one batch, with the full 2048-token K/V context. No collectives needed:
the only cross-shard dependency is LN1(x) over the full sequence, which each
core recomputes locally (trivial cost).

Host-side prep per core:
  - tokens permuted so the core's query half is rows 0..1023 (attention is
    permutation-invariant over keys when mask/V are permuted consistently)
  - all large operands pre-tiled to the on-chip [partition, ...] layout so
    every DMA is per-partition-contiguous (no descriptor storms)
  - enc^T precomputed (lhsT for cross-attention scores)
  - V packed as [v_h | 1] per head: the ones column makes the attn@V matmul
    emit the softmax denominator for free (row 64 of the PSUM accumulator)
  - mask converted to bf16 0/1, transposed to [key, query]

On-chip layout: scores are computed transposed ([key, query]) so the exp'd
scores feed the attn@V matmul directly as the moving operand - no transpose
of the 2048x1024 attention matrix is ever needed. Q/K and FFN matmuls run as
float32r (fp32 at 1 PE cycle/row for free-size >= 256); softmaxed scores and
V run bf16. Per-head output plus its softmax-denominator row are transposed
back to token-major together; normalization happens in the merge step as a
per-token reciprocal-multiply.
"""

import functools

import numpy as np

B = 4
S = 2048
D = 512
H = 8
DK = 64
DFF = 1024
Q = 1024  # queries per core
P = 128
TS = S // P  # 16 key tiles
TQ = Q // P  # 8 query tiles
EPS = 1e-5
N_CORES = 8


@functools.lru_cache(maxsize=None)
def _build_graph(a1, a2, a3, use_b1, use_b2, repeat=1, no_dma=False):
    """Build the (SPMD, per-core) Bass graph. aN: apply lnN gamma/beta.

    repeat>1 wraps the whole computation in an on-device loop (benchmarking
    aid: slope between repeat values isolates per-iteration exec time from
    the ~70ms PJRT/axon dispatch overhead)."""
    from contextlib import ExitStack

    import concourse.bacc as bacc
    import concourse.mybir as mybir
    import concourse.tile as tile
    from concourse.masks import make_identity

    f32 = mybir.dt.float32
    f32r = mybir.dt.float32r
    bf16 = mybir.dt.bfloat16
    AF = mybir.ActivationFunctionType
    OP = mybir.AluOpType

    nc = bacc.Bacc("TRN2", target_bir_lowering=False, debug=False)

    x_d = nc.dram_tensor("x", [S, D], f32, kind="ExternalInput")
    xq_d = nc.dram_tensor("xq", [P, TQ, D], f32, kind="ExternalInput")
    encT_d = nc.dram_tensor("encT", [P, 4, S], f32r, kind="ExternalInput")
    encv_d = nc.dram_tensor("encv", [P, TS, H, DK + 1], bf16, kind="ExternalInput")
    maskT_d = nc.dram_tensor("maskT", [P, TS, Q], bf16, kind="ExternalInput")
    w1_d = nc.dram_tensor("w1", [P, 4, DFF], f32r, kind="ExternalInput")
    w2_d = nc.dram_tensor("w2", [P, DFF // P, D], f32r, kind="ExternalInput")
    ln_d = {}
    for i, a in ((1, a1), (2, a2), (3, a3)):
        if a:
            ln_d[i] = (
                nc.dram_tensor(f"ln{i}gr", [P, D], f32, kind="ExternalInput"),
                nc.dram_tensor(f"ln{i}br", [P, D], f32, kind="ExternalInput"),
            )
    if use_b1:
        b1t_d = nc.dram_tensor("b1t", [P, DFF // P], f32, kind="ExternalInput")
    if use_b2:
        b2r_d = nc.dram_tensor("b2r", [P, D], f32, kind="ExternalInput")
    out_d = nc.dram_tensor("out", [P, TQ, D], f32, kind="ExternalOutput")

    with tile.TileContext(nc) as tc, ExitStack() as ctx:
        const = ctx.enter_context(tc.tile_pool(name="const", bufs=1))
        big = ctx.enter_context(tc.tile_pool(name="big", bufs=1))
        work = ctx.enter_context(tc.tile_pool(name="work", bufs=4))
        work3 = ctx.enter_context(tc.tile_pool(name="work3", bufs=4))
        xcpool = ctx.enter_context(tc.tile_pool(name="xcpool", bufs=1))
        work2 = ctx.enter_context(tc.tile_pool(name="work2", bufs=1))
        pssc = ctx.enter_context(tc.tile_pool(name="pssc", bufs=2, space="PSUM"))
        psacc = ctx.enter_context(tc.tile_pool(name="psacc", bufs=2, space="PSUM"))

        identf = const.tile([P, P], f32)
        make_identity(nc, identf)
        identb = const.tile([DK, DK], bf16)
        make_identity(nc, identb)
        epst = const.tile([P, 1], f32)
        nc.vector.memset(epst, EPS)
        ones = const.tile([P, 1], f32)
        nc.vector.memset(ones, 1.0)
        ln_sb = {}
        for i, (gd, bd) in ln_d.items():
            g = const.tile([P, D], f32, tag=f"ln{i}g")
            b = const.tile([P, D], f32, tag=f"ln{i}b")
            nc.sync.dma_start(g, gd.ap())
            nc.sync.dma_start(b, bd.ap())
            ln_sb[i] = (g, b)
        if use_b1:
            b1t = const.tile([P, DFF // P], f32)
            nc.sync.dma_start(b1t, b1t_d.ap())
        if use_b2:
            b2r = const.tile([P, D], f32)
            nc.sync.dma_start(b2r, b2r_d.ap())

        def _emit_iteration():
            # persistent / phase-aliased tensors (same tag = same SBUF slot)
            x1T = big.tile([P, 4, S], f32r, tag="A")            # LN1(x)^T
            v_self = big.tile([P, TS, H, DK + 1], bf16, tag="B")
            maskT = big.tile([P, TS, Q], bf16, tag="C")
            xbuf = big.tile([P, TQ, D], f32, tag="X")          # evolving query rows
            oT = big.tile([DK, H, Q], bf16, tag="OT")          # normalized attn out^T

            encT = big.tile([P, 4, S], f32r, tag="ENCT")
            encv = big.tile([P, TS, H, DK + 1], bf16, tag="ENCV")
            # stream x in 4 big chunks (all slots up front, so the bulk
            # loads below stay strictly behind them on the SP queue); the
            # first half of the tokens is the query block = xbuf itself
            x_cs = []
            for c in range(2):
                x_c = xcpool.tile([P, 4, D], f32, tag=f"xc{c}")
                x_cs.append(x_c)
            if no_dma:
                nc.gpsimd.memset(xbuf[:], 0.01)
                for c in range(2):
                    nc.gpsimd.memset(x_cs[c][:], 0.01)
                nc.gpsimd.memset(maskT[:], 1.0)
                nc.gpsimd.memset(encT[:].bitcast(f32), 0.01)
                nc.gpsimd.memset(encv[:], 0.01)
            else:
                nc.sync.dma_start(xbuf[:, 0:4, :], xq_d.ap()[:, 0:4, :])
                nc.sync.dma_start(xbuf[:, 4:8, :], xq_d.ap()[:, 4:8, :])
                for c in range(2):
                    nc.sync.dma_start(
                        x_cs[c], x_d.ap()[(c + 2) * 4 * P : (c + 3) * 4 * P].rearrange(
                            "(t p) d -> p t d", p=P
                        )
                    )
                nc.sync.dma_start(maskT, maskT_d.ap())
                nc.sync.dma_start(encT, encT_d.ap())
                nc.sync.dma_start(encv, encv_d.ap())
            nc.vector.tensor_copy(
                v_self[:, :, :, DK : DK + 1],
                ones[:, None, None, :].to_broadcast([P, TS, H, 1]),
            )

            def layer_norm_tile(x_t, which, use_act=False):
                """LN of a [P, D] tile (tokens on partitions) -> new f32 tile.

                use_act: compute the moments on the scalar engine (Copy/Square
                with accum_out) instead of DVE bn_stats - lets alternating
                tiles use different engines in the DVE-bound merge phases."""
                xn = work3.tile([P, D], f32, tag="xn")
                if not use_act:
                    stats = work.tile([P, 6], f32, tag="stats")
                    nc.vector.bn_stats(stats, x_t)
                    mv = work.tile([P, 2], f32, tag="mv")
                    nc.vector.bn_aggr(mv, stats)
                    mu = mv[:, 0:1]
                    var = mv[:, 1:2]
                else:
                    sums = work.tile([P, 1], f32, tag="sums")
                    nc.scalar.activation(xn, x_t, AF.Copy, accum_out=sums)
                    sumsq = work.tile([P, 1], f32, tag="sumsq")
                    nc.scalar.activation(xn, x_t, AF.Square, accum_out=sumsq)
                    mu = work.tile([P, 1], f32, tag="mu")
                    nc.vector.tensor_scalar_mul(mu, sums, 1.0 / D)
                    musq = work.tile([P, 1], f32, tag="musq")
                    nc.vector.tensor_mul(musq, mu, mu)
                    var = work.tile([P, 1], f32, tag="var")
                    nc.vector.tensor_scalar(
                        var, sumsq, scalar1=1.0 / D, scalar2=musq,
                        op0=OP.mult, op1=OP.subtract,
                    )
                rstd = work.tile([P, 1], f32, tag="rstd")
                nc.scalar.activation(rstd, var, AF.Sqrt, bias=epst[:])
                nc.vector.reciprocal(rstd, rstd)
                nc.vector.tensor_scalar(
                    xn, x_t, scalar1=mu, scalar2=rstd,
                    op0=OP.subtract, op1=OP.mult,
                )
                if which in ln_sb:
                    g, b = ln_sb[which]
                    nc.vector.tensor_mul(xn, xn, g)
                    nc.vector.tensor_add(xn, xn, b)
                return xn

            def transpose_to(dst, src_t, idx):
                """PE-transpose a [P, D] token-major tile into dst[:, :, idx*P:]."""
                pst = pssc.tile([P, 4, P], f32, tag="sc")
                for f in range(4):
                    nc.tensor.transpose(
                        pst[:, f, :], src_t[:, f * P : (f + 1) * P], identf[:]
                    )
                nc.scalar.copy(dst[:, :, idx * P : (idx + 1) * P], pst)

            # ---- phase 1: LN1 over all 16 token tiles; build x1T and V_self
            for t in range(TS):
                if t < 8:
                    x_t = xbuf[:, t, :]
                else:
                    x_t = x_cs[(t - 8) // 4][:, (t - 8) % 4, :]
                x1_t = layer_norm_tile(x_t, 1)
                nc.gpsimd.tensor_copy(
                    v_self[:, t, :, 0:DK], x1_t[:].rearrange("p (h d) -> p h d", h=H)
                )
                transpose_to(x1T, x1_t, t)

            def attn_block(kT, vv_all, qT, apply_mask, post_qt=None):
                """One attention block; accumulates result into xbuf.

                Heads are processed in interleaved pairs: two independent
                scores->exp->mask->attn@V chains keep every engine busy while
                the other chain's cross-engine semaphores propagate."""
                for h0 in range(0, H, 2):
                    accs = [
                        psacc.tile([DK + 1, Q], f32, tag="acc", name=f"acc{h0}"),
                        psacc.tile([DK + 1, Q], f32, tag="acc", name=f"acc{h0 + 1}"),
                    ]
                    # two interleaved head-chains: each engine always has an
                    # independent chain to work on while the other chain's
                    # cross-engine semaphores propagate
                    for s in range(TS):
                        for j in range(2):
                            h = h0 + j
                            hp = (h % 2) * DK
                            hf = h // 2
                            qTr = qT[hp : hp + DK, hf, 0:Q]
                            sc = pssc.tile([P, Q], f32, tag="sc", name=f"sc{j}")
                            kTr = kT[hp : hp + DK, hf, s * P : (s + 1) * P]
                            for n2 in range(2):
                                nc.tensor.matmul(
                                    sc[:, n2 * 512 : (n2 + 1) * 512],
                                    lhsT=kTr,
                                    rhs=qTr[:, n2 * 512 : (n2 + 1) * 512],
                                    start=True, stop=True,
                                )
                            at = work3.tile([P, Q], bf16, tag="attnT", name=f"at{j}")
                            nc.scalar.activation(at, sc[:], AF.Exp, scale=0.125)
                            if apply_mask:
                                nc.vector.tensor_mul(at, at, maskT[:, s, :])
                            for n2 in range(2):
                                nc.tensor.matmul(
                                    accs[j][:, n2 * 512 : (n2 + 1) * 512],
                                    lhsT=vv_all[:, s, h, :],
                                    rhs=at[:, n2 * 512 : (n2 + 1) * 512],
                                    start=(s == 0), stop=(s == TS - 1),
                                )
                    for j in range(2):
                        # epilogue (hidden under the next pair's compute):
                        # normalize rows 0..63 by row 64 (sum of exp)
                        acc = accs[j]
                        srow = work2.tile([1, Q], f32, tag="srow")
                        nc.vector.reciprocal(srow, acc[DK : DK + 1, :])
                        rcpB = work2.tile([DK, Q], f32, tag="rcpB")
                        nc.gpsimd.partition_broadcast(rcpB, srow)
                        nc.vector.tensor_mul(oT[:, h0 + j, :], acc[0:DK, :], rcpB)
                # merge heads back to token-major and add the residual
                for qt in range(TQ):
                    pso = pssc.tile([P, H, DK], bf16, tag="sc")
                    for h in range(H):
                        nc.tensor.transpose(
                            pso[:, h, :],
                            oT[:, h, qt * P : (qt + 1) * P],
                            identb[:],
                        )
                    nc.vector.tensor_add(
                        xbuf[:, qt, :],
                        xbuf[:, qt, :],
                        pso[:].rearrange("p h d -> p (h d)"),
                    )
                    if post_qt is not None:
                        post_qt(qt, None)

            # ---- phase 2: masked self-attention (merge pipelines into LN2)
            x2T_box = []

            def post_self(qt, sums):
                if not x2T_box:
                    x2T_box.append(big.tile([P, 4, Q], f32r, tag="A", name="x2T"))
                x2_t = layer_norm_tile(xbuf[:, qt], 2)
                transpose_to(x2T_box[0], x2_t, qt)

            attn_block(x1T, v_self, x1T, apply_mask=True, post_qt=post_self)

            # ---- phase 3: cross-attention (merge pipelines into LN3)
            x3T_box = []

            def post_cross(qt, sums):
                if not x3T_box:
                    x3T_box.append(big.tile([P, 4, Q], f32r, tag="B", name="x3T"))
                x3_t = layer_norm_tile(xbuf[:, qt], 3)
                transpose_to(x3T_box[0], x3_t, qt)

            attn_block(encT, encv, x2T_box[0], apply_mask=False, post_qt=post_cross)

            # ---- phase 4: FFN
            w1sb = big.tile([P, 4, DFF], f32r, tag="A")
            w2sb = big.tile([P, DFF // P, D], f32r, tag="OT")
            if no_dma:
                nc.gpsimd.memset(w1sb[:].bitcast(f32), 0.01)
                nc.gpsimd.memset(w2sb[:].bitcast(f32), 0.01)
            else:
                nc.sync.dma_start(w1sb, w1_d.ap())
                nc.sync.dma_start(w2sb, w2_d.ap())
            x3T = x3T_box[0]
            hT = big.tile([P, DFF // P, Q], f32r, tag="C")
            for f in range(DFF // P):
                for n2 in range(2):
                    hps = pssc.tile([P, 512], f32, tag="sc", name=f"hps{n2}")
                    for ft in range(4):
                        nc.tensor.matmul(
                            hps[:],
                            lhsT=w1sb[:, ft, f * P : (f + 1) * P],
                            rhs=x3T[:, ft, n2 * 512 : (n2 + 1) * 512],
                            start=(ft == 0), stop=(ft == 3),
                        )
                    bias = b1t[:, f : f + 1] if use_b1 else 0.0
                    nc.vector.tensor_scalar(
                        hT[:, f, n2 * 512 : (n2 + 1) * 512], hps[:],
                        scalar1=bias, scalar2=0.0, op0=OP.add, op1=OP.max,
                    )
            for qt in range(TQ):
                ops = pssc.tile([P, D], f32, tag="sc")
                for f in range(DFF // P):
                    nc.tensor.matmul(
                        ops[:],
                        lhsT=hT[:, f, qt * P : (qt + 1) * P],
                        rhs=w2sb[:, f, :],
                        start=(f == 0), stop=(f == DFF // P - 1),
                    )
                nc.vector.tensor_add(xbuf[:, qt], xbuf[:, qt], ops)
                if use_b2:
                    nc.vector.tensor_add(xbuf[:, qt], xbuf[:, qt], b2r)
                nc.sync.dma_start(out_d.ap()[:, qt], xbuf[:, qt])

        if repeat == 1:
            _emit_iteration()
        else:
            with tc.For_i(0, repeat, 1):
                _emit_iteration()

    nc.compile()
    return nc


def _tile_p(a, inner=P):
    """[N*P, ...] -> [P, N, ...] so each SBUF partition's data is contiguous."""
    return np.ascontiguousarray(
        a.reshape(a.shape[0] // inner, inner, *a.shape[1:]).swapaxes(0, 1)
    )


def _prep_core_inputs(x, encoder_output, mask, W1, b1, W2, b2, ln_aff, flags):
    """Build per-core in_maps (host-side sharding + layout prep)."""
    import ml_dtypes

    a1, a2, a3, use_b1, use_b2 = flags
    in_maps = []
    for c in range(N_CORES):
        b, half = c // 2, c % 2
        q0 = half * Q
        perm = np.concatenate(
            [np.arange(q0, q0 + Q), np.arange((1 - half) * Q, (1 - half) * Q + Q)]
        )
        xb = np.ascontiguousarray(x[b][perm]).astype(np.float32)
        enc = encoder_output[b].astype(np.float32)
        encT = np.ascontiguousarray(enc.T)
        encv = np.empty((S, H, DK + 1), ml_dtypes.bfloat16)
        encv[:, :, :DK] = enc.reshape(S, H, DK).astype(ml_dtypes.bfloat16)
        encv[:, :, DK] = 1.0
        m = mask[b, 0][q0 : q0 + Q][:, perm]  # [Q, S] in permuted key order
        maskT = np.ascontiguousarray(m.T).astype(ml_dtypes.bfloat16)
        im = {
            "x": xb,
            "xq": _tile_p(xb[0:Q]),
            "encT": _tile_p(encT),
            "encv": _tile_p(encv),
            "maskT": _tile_p(maskT),
            "w1": _tile_p(W1.astype(np.float32)),
            "w2": _tile_p(W2.astype(np.float32)),
        }
        for i, a in ((1, a1), (2, a2), (3, a3)):
            if a:
                g, bta = ln_aff[i]
                im[f"ln{i}gr"] = np.tile(g.astype(np.float32)[None, :], (P, 1))
                im[f"ln{i}br"] = np.tile(bta.astype(np.float32)[None, :], (P, 1))
        if use_b1:
            im["b1t"] = np.ascontiguousarray(
                b1.astype(np.float32).reshape(DFF // P, P).T
            )
        if use_b2:
            im["b2r"] = np.tile(b2.astype(np.float32)[None, :], (P, 1))
        in_maps.append(im)
    return in_maps


def kernel(x, encoder_output, mask, ln1_g, ln1_b, ln2_g, ln2_b, ln3_g, ln3_b,
           W1, b1, W2, b2):
    from concourse import bass_utils

    x = np.asarray(x)
    encoder_output = np.asarray(encoder_output)
    mask = np.asarray(mask)
    ln = {
        1: (np.asarray(ln1_g), np.asarray(ln1_b)),
        2: (np.asarray(ln2_g), np.asarray(ln2_b)),
        3: (np.asarray(ln3_g), np.asarray(ln3_b)),
    }
    flags = (
        *(not (np.all(ln[i][0] == 1.0) and np.all(ln[i][1] == 0.0)) for i in (1, 2, 3)),
        bool(np.any(np.asarray(b1) != 0.0)),
        bool(np.any(np.asarray(b2) != 0.0)),
    )
    nc = _build_graph(*flags)
    in_maps = _prep_core_inputs(
        x, encoder_output, mask, np.asarray(W1), np.asarray(b1), np.asarray(W2),
        np.asarray(b2), ln, flags,
    )
    res = bass_utils.run_bass_kernel_spmd(nc, in_maps, core_ids=list(range(N_CORES)))
    out = np.empty((B, S, D), np.float32)
    for c in range(N_CORES):
        b, half = c // 2, c % 2
        # out dram layout is [P, TQ, D] -> token-major [Q, D]
        o = res.results[c]["out"].swapaxes(0, 1).reshape(Q, D)
        out[b, half * Q : (half + 1) * Q] = o
    return out

